# revision 9
# baseline (speedup 1.0000x reference)
"""Persistent-homology loss (coefficient of variation of the pairwise
distance matrix) on 8 TRN2 NeuronCores.

Math:
  X = embeddings.reshape(8192, 128)
  d2_ij = sq_i + sq_j - 2 X_i.X_j   (clamped >= 0), d = sqrt(d2)
  out = std(d, ddof=1) / (mean(d) + 1e-8) over all N^2 entries.

Split:
  S2 = sum(d2) exactly on host in f64 via 2N*sum(sq) - 2*||sum(X)||^2.
  S1_diag = sum of d over the 64 diagonal 128x128 chunk blocks, on host
  in f64 (1/64 of the entries; avoids the clamp-at-zero problem and the
  DVE engine entirely).
  S1_off = everything else on device (the N^2 work).

Device strategy (SPMD, one program, per-core data):
  - 8192 points in 64 chunks of 128; core c owns 8 chunks (1024 rows).
  - Each input is rolled by -1024c so the core's own rows sit at local
    cols [0, 1024): all offsets are program-uniform.
  - Circulant half-window: for own chunk rc (local cols [128rc, +128)),
    cover cols [128rc+128, 128rc+4096) at weight 2 and the delta-32
    chunk [128rc+4096, +128) at weight 1 (it is computed once from each
    side). Every unordered off-diagonal chunk pair gets total weight 2.
  - PSUM[i,j] = X_i.X_j - 0.5 sq_i - 0.5 sq_j: main bf16 matmul plus a
    K=4 aux bf16 matmul (lhsT rows = [a_hi_i, a_lo_i, 1, 1], rhs rows =
    [1, 1, a_hi_j, a_lo_j], a = -0.5 sq split into bf16 hi+lo).
  - ACT computes sqrt(psum * -2) with accum_out per-partition row sums
    into `partials` [128, 24]. min off-diag d2 is ~89 so no clamping.
  - Everything rides ONE input DMA + ONE output DMA: walrus allows only
    one semaphore wait per compute instruction and a small number on the
    closing drain, so the program must funnel through few semaphores
    (PE, ACT, and two DMA queues).
"""

import numpy as np
import ml_dtypes

import concourse.bass as bass
import concourse.tile as tile
from concourse import mybir
from concourse.bass_utils import run_bass_kernel_spmd

BF16 = ml_dtypes.bfloat16
N = 8192
D = 128
NCORES = 8
LOCAL = N // NCORES          # 1024 rows per core
NCHUNK = LOCAL // 128        # 8 row-chunks per core
XT_COLS = 5120               # max col touched: 7*128 + 128 + 4096
AUXL = XT_COLS               # [4, 1024]: a_hi/a_lo/1/1 for local rows
AUXR = AUXL + LOCAL - 128    # rhs col c maps to blob col AUXR + c
TOT = AUXR + XT_COLS         # 11136

F32 = mybir.dt.float32
MBF16 = mybir.dt.bfloat16
SQRT = mybir.ActivationFunctionType.Sqrt


def _build_nc() -> bass.Bass:
    nc = bass.Bass()
    blob_d = nc.declare_dram_parameter("blob", [D, TOT], MBF16, isOutput=False)
    out_d = nc.declare_dram_parameter("out", [D, 24], F32, isOutput=True)

    with tile.TileContext(nc) as tc:
        with (
            tc.tile_pool(name="sb", bufs=1) as sb,
            tc.tile_pool(name="sc", bufs=2) as sc,
            tc.tile_pool(name="pb", bufs=2, space="PSUM") as pbp,
        ):
            blob = sb.tile([D, TOT], MBF16)
            partials = sb.tile([D, 24], F32)
            nc.sync.dma_start(blob[:], blob_d[:])

            for rc in range(NCHUNK):
                r0 = rc * 128
                for h in range(2):
                    base = r0 + 128 + 2048 * h
                    pb = pbp.tile([D, 2048], F32, name=f"pb_{rc}_{h}", tag="pb")
                    for s in range(4):
                        c0 = base + 512 * s
                        cs = 512 * s
                        nc.tensor.matmul(
                            pb[:, cs:cs + 512],
                            blob[:, r0:r0 + 128], blob[:, c0:c0 + 512],
                            start=True, stop=False,
                        )
                        nc.tensor.matmul(
                            pb[:, cs:cs + 512],
                            blob[0:4, AUXL + r0:AUXL + r0 + 128],
                            blob[0:4, AUXR + c0:AUXR + c0 + 512],
                            start=False, stop=True,
                        )
                    ob = sc.tile([D, 2048], F32, name=f"ob_{rc}_{h}", tag="ob")
                    if h == 0:
                        nc.scalar.activation(
                            ob[:], pb[:], SQRT, scale=-2.0,
                            accum_out=partials[:, 2 * rc:2 * rc + 1],
                        )
                    else:
                        nc.scalar.activation(
                            ob[:, :1920], pb[:, :1920], SQRT, scale=-2.0,
                            accum_out=partials[:, 2 * rc + 1:2 * rc + 2],
                        )
                        nc.scalar.activation(
                            ob[:, 1920:], pb[:, 1920:], SQRT, scale=-2.0,
                            accum_out=partials[:, 16 + rc:17 + rc],
                        )

            nc.sync.dma_start(out_d[:], partials[:])

    # Walrus allows at most ONE semaphore wait per compute instruction and
    # per Drain. Two prunes, both semantically safe:
    #  - same-engine waits (ACT waiting Activation_*, Matmult waiting PE_*)
    #    are implied by in-order engine queues;
    #  - the closing Drain only needs the out-DMA queue sem, which
    #    transitively dominates everything (out-DMA <- ACT <- PE <- in-DMA).
    same = {"Activation": "Activation", "Matmult": "PE", "Ldweights": "PE"}
    for inst in nc.all_instructions():
        si = inst.sync_info
        if not si or not si.on_wait:
            continue
        if inst.opcode == "Drain" and len(si.on_wait) > 1:
            keep = [w for w in si.on_wait if w.ant_name.startswith("DMAHW1")]
            assert len(keep) == 1, [w.ant_name for w in si.on_wait]
            si.on_wait[:] = keep
            continue
        pref = same.get(inst.opcode)
        if pref and len(si.on_wait) > 1:
            keep = [w for w in si.on_wait if not w.ant_name.startswith(pref)]
            assert len(keep) == 1, (inst.name, [w.ant_name for w in si.on_wait])
            si.on_wait[:] = keep
    return nc


def _host_prep(embeddings: np.ndarray):
    x = np.ascontiguousarray(embeddings.reshape(N, D).astype(np.float32))
    xT = np.ascontiguousarray(x.T)                      # [128, 8192] f32
    x64 = x.astype(np.float64)
    sq64 = np.einsum("ij,ij->i", x64, x64)              # [8192]
    ssum = x64.sum(axis=0)                              # [128]
    S2 = 2.0 * N * sq64.sum() - 2.0 * float(ssum @ ssum)

    S1_diag = 0.0
    for g in range(N // 128):
        Xg = x64[128 * g:128 * g + 128]
        sg = sq64[128 * g:128 * g + 128]
        d2 = sg[:, None] + sg[None, :] - 2.0 * (Xg @ Xg.T)
        np.maximum(d2, 0.0, out=d2)
        S1_diag += float(np.sqrt(d2).sum())

    a32 = (-0.5 * sq64).astype(np.float32)
    a_hi = a32.astype(BF16)
    a_lo = (a32 - a_hi.astype(np.float32)).astype(BF16)

    in_maps = []
    for c in range(NCORES):
        sh = -LOCAL * c
        xt_c = np.roll(xT, sh, axis=1)[:, :XT_COLS]
        hi_r = np.roll(a_hi, sh)
        lo_r = np.roll(a_lo, sh)
        blob = np.zeros((D, TOT), BF16)
        blob[:, :XT_COLS] = xt_c.astype(BF16)
        blob[0, AUXL:AUXL + LOCAL] = hi_r[:LOCAL]
        blob[1, AUXL:AUXL + LOCAL] = lo_r[:LOCAL]
        blob[2:4, AUXL:AUXL + LOCAL] = 1
        blob[0:2, AUXR + 128:AUXR + XT_COLS] = 1
        blob[2, AUXR + 128:AUXR + XT_COLS] = hi_r[128:XT_COLS]
        blob[3, AUXR + 128:AUXR + XT_COLS] = lo_r[128:XT_COLS]
        in_maps.append({"blob": blob})
    return in_maps, S2, S1_diag


def _combine(parts: list[np.ndarray], S2: float, S1_diag: float) -> np.ndarray:
    S1 = S1_diag
    for p in parts:
        p = p.astype(np.float64)
        S1 += 2.0 * p[:, :16].sum() + p[:, 16:24].sum()
    NN = float(N) * float(N)
    mean = S1 / NN
    var = (S2 - NN * mean * mean) / (NN - 1.0)
    return np.float32(np.sqrt(max(var, 0.0)) / (mean + 1e-8))


_NC_CACHE = None


def kernel(embeddings: np.ndarray) -> np.ndarray:
    global _NC_CACHE
    in_maps, S2, S1_diag = _host_prep(embeddings)
    if _NC_CACHE is None:
        _NC_CACHE = _build_nc()
    res = run_bass_kernel_spmd(_NC_CACHE, in_maps, list(range(NCORES)))
    return _combine([r["out"] for r in res.results], S2, S1_diag)


# revision 12
# speedup vs baseline: 2.0285x; 2.0285x over previous
"""Persistent-homology loss (coefficient of variation of the pairwise
distance matrix) on 8 TRN2 NeuronCores.

Math:
  X = embeddings.reshape(8192, 128)
  d2_ij = sq_i + sq_j - 2 X_i.X_j   (min off-diag d2 ~ 89, no clamp), d = sqrt(d2)
  out = std(d, ddof=1) / (mean(d) + 1e-8) over all N^2 entries.

Split:
  S2 = sum(d2) exactly on host in f64 via 2N*sum(sq) - 2*||sum(X)||^2.
  S1_diag = sum of d over the 64 diagonal 128x128 chunk blocks, host f64.
  S1_off = everything else on device.

Device strategy (SPMD, one program, per-core rolled data):
  - 8192 points in 64 chunks of 128; core c owns 8 chunks (1024 rows),
    inputs rolled by -1024c so offsets are program-uniform.
  - Circulant half-window per own chunk rc: cols [r0+128, r0+4096) at
    weight 2, [r0+4096, r0+4224) at weight 1.
  - PE: 64 single-shot N=512 bf16 matmuls u = X_i.X_j (start+stop, no
    PSUM read-modify-write -> ~380ns each).
  - DVE (vector for even rc, gpsimd for odd): one scalar_tensor_tensor
    per 2048-col block: (u - 0.5 sq_i) - 0.5 sq_j into an SBUF tmp tile.
  - ACT: sqrt(tmp * -2) with accum_out per-partition row sums into
    partials [128, 24] (per rc: 2048 w2 / 1920 w2 / 128 w1 splits).
  - Sync: walrus allows ONE semaphore wait per compute instruction.
    Per-engine warmup reads establish the sq-DMA watermark; same-engine
    waits are stripped post-build (in-order queues imply them); the
    closing Drain keeps only the out-DMA queue sem which transitively
    dominates everything.
"""

import numpy as np
import ml_dtypes

import concourse.bass as bass
import concourse.tile as tile
from concourse import mybir
from concourse.bass_utils import run_bass_kernel_spmd

BF16 = ml_dtypes.bfloat16
N = 8192
D = 128
NCORES = 8
LOCAL = N // NCORES          # 1024 rows per core
NCHUNK = LOCAL // 128        # 8 row-chunks per core
XT_COLS = 5120               # max col touched: 7*128 + 128 + 4096
SQ_COLS = XT_COLS + NCHUNK   # + per-partition 0.5*sq_i column per chunk

F32 = mybir.dt.float32
MBF16 = mybir.dt.bfloat16
SQRT = mybir.ActivationFunctionType.Sqrt
ALU = mybir.AluOpType


def _build_nc() -> bass.Bass:
    nc = bass.Bass()
    xt_d = nc.declare_dram_parameter("xt", [D, XT_COLS], MBF16, isOutput=False)
    sq_d = nc.declare_dram_parameter("sq", [D, SQ_COLS], F32, isOutput=False)
    out_d = nc.declare_dram_parameter("out", [D, 24], F32, isOutput=True)

    with tile.TileContext(nc) as tc:
        with (
            tc.tile_pool(name="sb", bufs=1) as sb,
            tc.tile_pool(name="ob", bufs=2) as obp,
            tc.tile_pool(name="pp", bufs=2, space="PSUM") as pp,
        ):
            xt = sb.tile([D, XT_COLS], MBF16)
            sq = sb.tile([D, SQ_COLS], F32)
            partials = sb.tile([D, 24], F32)
            wv = sb.tile([1, 1], F32, name="wv")
            nc.sync.dma_start(xt[:], xt_d[:])
            nc.sync.dma_start(sq[:], sq_d[:])
            # establish the sq-DMA watermark on the vector engine
            # (gpsimd cannot access PSUM, so ALL STTs run on vector)
            nc.vector.tensor_scalar_add(wv[0:1, 0:1], sq[0:1, 0:1], 0.0)

            tmps = [sb.tile([D, 4096], F32, name=f"tmp{rc}")
                    for rc in range(NCHUNK)]
            for rc in range(NCHUNK):
                r0 = rc * 128
                eng = nc.vector
                for h in range(2):
                    base = r0 + 128 + 2048 * h
                    ps = pp.tile([D, 2048], F32, name=f"ps_{rc}_{h}", tag="ps")
                    for s in range(4):
                        c0 = base + 512 * s
                        nc.tensor.matmul(
                            ps[:, 512 * s:512 * s + 512],
                            xt[:, r0:r0 + 128], xt[:, c0:c0 + 512],
                            start=True, stop=True,
                        )
                    eng.scalar_tensor_tensor(
                        tmps[rc][:, 2048 * h:2048 * h + 2048], ps[:],
                        sq[:, XT_COLS + rc:XT_COLS + rc + 1],
                        sq[:, base:base + 2048],
                        ALU.subtract, ALU.subtract,
                    )
                o = obp.tile([D, 4096], MBF16, name=f"o{rc}", tag="o")
                nc.scalar.activation(
                    o[:, 0:2048], tmps[rc][:, 0:2048], SQRT, scale=-2.0,
                    accum_out=partials[:, 2 * rc:2 * rc + 1],
                )
                nc.scalar.activation(
                    o[:, 2048:3968], tmps[rc][:, 2048:3968], SQRT, scale=-2.0,
                    accum_out=partials[:, 2 * rc + 1:2 * rc + 2],
                )
                nc.scalar.activation(
                    o[:, 3968:4096], tmps[rc][:, 3968:4096], SQRT, scale=-2.0,
                    accum_out=partials[:, 16 + rc:17 + rc],
                )

            nc.sync.dma_start(out_d[:], partials[:])

    # Walrus allows at most ONE semaphore wait per compute instruction and
    # per Drain. Same-engine waits are implied by in-order engine queues;
    # the closing Drain only needs the out-DMA queue sem (out-DMA <- ACT
    # <- DVE <- PE <- in-DMAs, and DVE warmups cover the sq DMA).
    same = {"Activation": "Activation", "Matmult": "PE", "Ldweights": "PE",
            "TensorScalarPtr": "Pool", "ScalarTensorTensor": "Pool"}
    for inst in nc.all_instructions():
        si = inst.sync_info
        if not si or not si.on_wait:
            continue
        if inst.opcode == "Drain" and len(si.on_wait) > 1:
            keep = [w for w in si.on_wait if w.ant_name.startswith("DMAHW")]
            si.on_wait[:] = keep[-1:]
            continue
        pref = same.get(inst.opcode)
        if pref and len(si.on_wait) > 1:
            keep = [w for w in si.on_wait if not w.ant_name.startswith(pref)]
            if keep:
                si.on_wait[:] = keep
    return nc


def _host_prep(embeddings: np.ndarray):
    x = np.ascontiguousarray(embeddings.reshape(N, D).astype(np.float32))
    xT = np.ascontiguousarray(x.T)                      # [128, 8192] f32
    x64 = x.astype(np.float64)
    sq64 = np.einsum("ij,ij->i", x64, x64)              # [8192]
    ssum = x64.sum(axis=0)                              # [128]
    S2 = 2.0 * N * sq64.sum() - 2.0 * float(ssum @ ssum)

    S1_diag = 0.0
    for g in range(N // 128):
        Xg = x64[128 * g:128 * g + 128]
        sg = sq64[128 * g:128 * g + 128]
        d2 = sg[:, None] + sg[None, :] - 2.0 * (Xg @ Xg.T)
        np.maximum(d2, 0.0, out=d2)
        S1_diag += float(np.sqrt(d2).sum())

    half_sq = (0.5 * sq64).astype(np.float32)           # [8192]

    in_maps = []
    for c in range(NCORES):
        sh = -LOCAL * c
        xt_c = np.ascontiguousarray(
            np.roll(xT, sh, axis=1)[:, :XT_COLS].astype(BF16))
        hs = np.roll(half_sq, sh)
        sqv = np.empty((D, SQ_COLS), np.float32)
        sqv[:, :XT_COLS] = hs[None, :XT_COLS]
        for rc in range(NCHUNK):
            sqv[:, XT_COLS + rc] = hs[128 * rc:128 * rc + 128]
        in_maps.append({"xt": xt_c, "sq": sqv})
    return in_maps, S2, S1_diag


def _combine(parts: list[np.ndarray], S2: float, S1_diag: float) -> np.ndarray:
    S1 = S1_diag
    for p in parts:
        p = p.astype(np.float64)
        S1 += 2.0 * p[:, :16].sum() + p[:, 16:24].sum()
    NN = float(N) * float(N)
    mean = S1 / NN
    var = (S2 - NN * mean * mean) / (NN - 1.0)
    return np.float32(np.sqrt(max(var, 0.0)) / (mean + 1e-8))


_NC_CACHE = None


def kernel(embeddings: np.ndarray) -> np.ndarray:
    global _NC_CACHE
    in_maps, S2, S1_diag = _host_prep(embeddings)
    if _NC_CACHE is None:
        _NC_CACHE = _build_nc()
    res = run_bass_kernel_spmd(_NC_CACHE, in_maps, list(range(NCORES)))
    return _combine([r["out"] for r in res.results], S2, S1_diag)
